# revision 27
# baseline (speedup 1.0000x reference)
"""Conv1D (B=32, L=8192, C_in=64, K=3, F=128, VALID) + bias + ReLU on 8 trn2 cores.

Strategy: data-parallel over batch (4 batches per core). Per core:
  - Casting SWDGE DMA loads x[b] position-chunks as bf16, t-major
    ([128, T*64] tiles; partition = position within each 128-block).
  - PE-transpose [128,128] bf16 sub-tiles; both PSUM halves copy out as
    contiguous [64,128] blocks into x_T [65, L] (row 64 = ones so the bias
    rides row 64 of the k=0 weights).
  - out[pos, F]: per 512-position PSUM bank, 4 interleaved-M matmul groups
    (position = g0 + 4*q + m) x 3 accumulated k-shifts; interleaving makes
    the output DMA's per-partition DRAM runs 2KB instead of 512B.
  - One ReLU (ScalarE) per bank -> SBUF staging -> big contiguous DMA out.
"""

import os
import sys

import numpy as np

_TRN_REPO = "/opt/trn_rl_repo"
if _TRN_REPO not in sys.path and os.path.isdir(_TRN_REPO):
    sys.path.insert(0, _TRN_REPO)

import concourse.bass as bass
import concourse.tile as tile
from concourse import bacc, mybir
from concourse.bass_utils import run_bass_kernel_spmd
from concourse.masks import make_identity

B, L, C = 32, 8192, 64
K, F = 3, 128
L_OUT = L - K + 1  # 8190
N_CORES = 8
B_SHARD = B // N_CORES  # 4

MM_DT = mybir.dt.bfloat16

IN_CHUNK = 1024  # positions per input DMA chunk
T_IN = IN_CHUNK // 128  # 16
G = 512  # positions per PSUM output bank
GI = 4  # M-interleave within a bank (G // 128)
OUT_CHUNK = 1024  # positions per output staging tile


def _conv_kernel(tc: tile.TileContext, out_ap, x_ap, w_ap, b_ap, mm_dt):
    nc = tc.nc
    fp32 = mybir.dt.float32

    with (
        tc.tile_pool(name="setup", bufs=1) as setup_pool,
        tc.tile_pool(name="xin", bufs=3) as xin_pool,
        tc.tile_pool(name="xbf", bufs=3) as xbf_pool,
        tc.tile_pool(name="osb", bufs=3) as osb_pool,
        tc.tile_pool(name="pt", bufs=4, space="PSUM") as pt_pool,
        tc.tile_pool(name="po", bufs=4, space="PSUM") as po_pool,
    ):
        # --- one-time setup: weights, bias, identity, xT double-buffer ---
        wstage = setup_pool.tile([C, K * F], fp32)
        for k in range(K):
            nc.sync.dma_start(out=wstage[:, k * F : (k + 1) * F], in_=w_ap[k])
        bstage = setup_pool.tile([1, F], fp32)
        nc.sync.dma_start(out=bstage[:, :], in_=b_ap[None, :])

        # w rows 0-63 = w[k]; row 64 of the k=0 slice = bias; rows 65-127
        # zero (pad to K=128 -> measurably faster matmuls).
        wpad = setup_pool.tile([128, K * F], mm_dt)
        nc.vector.memset(wpad[:, :], 0.0)
        nc.vector.tensor_copy(wpad[0:C, :], wstage[:, :])
        nc.vector.tensor_copy(wpad[C : C + 1, 0:F], bstage[:, :])

        ident = setup_pool.tile([128, 128], mm_dt)
        make_identity(nc, ident)

        # xT: manually double-buffered [128, 2*L]; row 64 ones and zero pad
        # rows 65-127 (K=128 matmuls). Pad rows are initialized piecewise
        # alongside the first two batches' fills so they don't gate startup.
        xT = setup_pool.tile([128, 2 * L], mm_dt)
        xT4 = xT.rearrange("c (n s) -> c n s", s=GI)

        for b in range(B_SHARD):
            half = (b % 2) * L

            def fill_chunk(b, ci, half):
                c0 = ci * IN_CHUNK
                xin = xin_pool.tile([128, T_IN * C], fp32, name=f"xin_{b}_{ci}", tag="xin")
                nc.sync.dma_start(
                    out=xin.rearrange("p (t c) -> p t c", c=C),
                    in_=x_ap[b, c0 : c0 + IN_CHUNK, :].rearrange(
                        "(t p) c -> p t c", p=128
                    ),
                )
                if b < 2:
                    c0h = half + c0
                    nc.vector.memset(xT[C:128, c0h : c0h + IN_CHUNK], 0.0)
                    nc.gpsimd.memset(xT[C : C + 1, c0h : c0h + IN_CHUNK], 1.0)
                xbf = xbf_pool.tile([128, T_IN * C], mm_dt, name=f"xbf_{b}_{ci}", tag="xbf")
                nc.gpsimd.tensor_copy(xbf[:, :], xin[:, :])
                # transpose [128,128] sub-tiles; sub-tile j holds t in {2j, 2j+1}
                for j in range(T_IN * C // 128):
                    pt = pt_pool.tile([128, 128], mm_dt, name=f"pt_{b}_{ci}_{j}", tag="pt")
                    nc.tensor.transpose(pt[:, :], xbf[:, j * 128 : (j + 1) * 128], ident)
                    for tt in range(2):
                        q = half + c0 + (2 * j + tt) * 128
                        nc.vector.tensor_copy(
                            xT[0:C, q : q + 128],
                            pt[tt * C : (tt + 1) * C, :],
                        )

            def store_chunk(b, oc, half):
                o0 = oc * OUT_CHUNK
                opos = min(OUT_CHUNK, L_OUT - o0)  # 2048 or 2046
                osb = osb_pool.tile([128, OUT_CHUNK], fp32, name=f"osb_{b}_{oc}", tag="osb")
                n_g = (opos + G - 1) // G
                for gc in range(n_g):
                    g0 = o0 + gc * G
                    gpos = min(G, L_OUT - g0)  # 512 or 510
                    po = po_pool.tile([128, G], fp32, name=f"po_{b}_{oc}_{gc}", tag="po")
                    n_sub = (gpos + 127) // 128
                    for m in range(GI):
                        Pm = (gpos - m + GI - 1) // GI  # positions g0+4q+m
                        for k in range(K):
                            mk = m + k
                            n0 = (half + g0 + mk) // GI
                            nc.tensor.matmul(
                                po[0:Pm, m * F : (m + 1) * F],
                                xT4[:, n0 : n0 + Pm, mk % GI],
                                wpad[:, k * F : (k + 1) * F],
                                start=(k == 0),
                                stop=(k == K - 1),
                            )
                    ob = gc * G
                    if gpos == G:
                        nc.scalar.activation(
                            osb[:, ob : ob + G],
                            po[:, :],
                            mybir.ActivationFunctionType.Relu,
                        )
                    else:
                        # tail bank (510): m=0,1 have 128 rows; m=2,3 have 127
                        nc.scalar.activation(
                            osb[:, ob : ob + 2 * F],
                            po[:, 0 : 2 * F],
                            mybir.ActivationFunctionType.Relu,
                        )
                        nc.scalar.activation(
                            osb[0:127, ob + 2 * F : ob + 4 * F],
                            po[0:127, 2 * F : 4 * F],
                            mybir.ActivationFunctionType.Relu,
                        )
                # store; alternate HWDGE rings
                eng = nc.scalar if (b * 4 + oc) % 2 == 0 else nc.sync
                if opos == OUT_CHUNK:
                    eng.dma_start(
                        out=out_ap[b, o0 : o0 + opos, :].rearrange(
                            "(pc q m) f -> q pc m f", q=128, m=GI
                        ),
                        in_=osb.rearrange("q (pc m f) -> q pc m f", m=GI, f=F),
                    )
                else:
                    nfull = (opos // G) * G
                    if nfull:
                        eng.dma_start(
                            out=out_ap[b, o0 : o0 + nfull, :].rearrange(
                                "(pc q m) f -> q pc m f", q=128, m=GI
                            ),
                            in_=osb[:, 0 : nfull].rearrange(
                                "q (pc m f) -> q pc m f", m=GI, f=F
                            ),
                        )
                    # tail group 510 = 127 q-groups of 4 + 2 leftover positions
                    t0 = o0 + nfull
                    eng.dma_start(
                        out=out_ap[b, t0 : t0 + 508, :].rearrange(
                            "(q m) f -> q m f", m=GI
                        ),
                        in_=osb[0:127, nfull : nfull + G].rearrange(
                            "q (m f) -> q m f", f=F
                        ),
                    )
                    eng.dma_start(
                        out=out_ap[b, t0 + 508 : t0 + 510, :].rearrange(
                            "p f -> (p f)"
                        )[None, :],
                        in_=osb[127:128, nfull : nfull + 2 * F],
                    )

            for ci in range(L // IN_CHUNK):
                fill_chunk(b, ci, half)
                if ci >= 1:
                    store_chunk(b, ci - 1, half)
            store_chunk(b, L // IN_CHUNK - 1, half)

def build_program(mm_dt=MM_DT):
    nc = bacc.Bacc("TRN2", target_bir_lowering=False, debug=False)
    x = nc.dram_tensor("x", [B_SHARD, L, C], mybir.dt.float32, kind="ExternalInput")
    w = nc.dram_tensor("w", [K, C, F], mybir.dt.float32, kind="ExternalInput")
    bb = nc.dram_tensor("b", [F], mybir.dt.float32, kind="ExternalInput")
    out = nc.dram_tensor(
        "out", [B_SHARD, L_OUT, F], mybir.dt.float32, kind="ExternalOutput"
    )
    with tile.TileContext(nc) as tc:
        _conv_kernel(tc, out.ap(), x.ap(), w.ap(), bb.ap(), mm_dt)
    nc.compile()
    return nc


def kernel(x, w, b, _trace=False, _trace_kwargs=None):
    x = np.ascontiguousarray(np.asarray(x, dtype=np.float32))
    w = np.ascontiguousarray(np.asarray(w, dtype=np.float32))
    b = np.ascontiguousarray(np.asarray(b, dtype=np.float32))
    assert x.shape == (B, L, C) and w.shape == (K, C, F) and b.shape == (F,)

    nc = build_program()
    in_maps = [
        {"x": x[i * B_SHARD : (i + 1) * B_SHARD], "w": w, "b": b}
        for i in range(N_CORES)
    ]
    res = run_bass_kernel_spmd(
        nc,
        in_maps,
        core_ids=list(range(N_CORES)),
        trace=_trace,
        **(_trace_kwargs or {}),
    )
    out = np.concatenate([r["out"] for r in res.results], axis=0)
    if _trace:
        return out, res
    return out


if __name__ == "__main__":
    rng = np.random.default_rng(0)
    x = rng.standard_normal((B, L, C), dtype=np.float32)
    w = rng.standard_normal((K, C, F), dtype=np.float32) * 0.08
    b = np.zeros((F,), dtype=np.float32)
    out = kernel(x, w, b)
    print("out", out.shape, out.dtype, float(np.abs(out).max()))


# revision 28
# speedup vs baseline: 1.1248x; 1.1248x over previous
"""Conv1D (B=32, L=8192, C_in=64, K=3, F=128, VALID) + bias + ReLU on 8 trn2 cores.

Strategy: data-parallel over batch (4 batches per core). Per core:
  - Casting SWDGE DMA loads x[b] position-chunks as bf16, t-major
    ([128, T*64] tiles; partition = position within each 128-block).
  - PE-transpose [128,128] bf16 sub-tiles; both PSUM halves copy out as
    contiguous [64,128] blocks into x_T [65, L] (row 64 = ones so the bias
    rides row 64 of the k=0 weights).
  - out[pos, F]: per 512-position PSUM bank, 4 interleaved-M matmul groups
    (position = g0 + 4*q + m) x 3 accumulated k-shifts; interleaving makes
    the output DMA's per-partition DRAM runs 2KB instead of 512B.
  - One ReLU (ScalarE) per bank -> SBUF staging -> big contiguous DMA out.
"""

import os
import sys

import numpy as np

_TRN_REPO = "/opt/trn_rl_repo"
if _TRN_REPO not in sys.path and os.path.isdir(_TRN_REPO):
    sys.path.insert(0, _TRN_REPO)

import concourse.bass as bass
import concourse.tile as tile
from concourse import bacc, mybir
from concourse.bass_utils import run_bass_kernel_spmd
from concourse.masks import make_identity

B, L, C = 32, 8192, 64
K, F = 3, 128
L_OUT = L - K + 1  # 8190
N_CORES = 8
B_SHARD = B // N_CORES  # 4

MM_DT = mybir.dt.bfloat16

IN_CHUNK = 1024  # positions per input DMA chunk
T_IN = IN_CHUNK // 128  # 16
G = 512  # positions per PSUM output bank
GI = 4  # M-interleave within a bank (G // 128)
OUT_CHUNK = 1024  # positions per output staging tile


def _conv_kernel(tc: tile.TileContext, out_ap, x_ap, w_ap, b_ap, mm_dt):
    nc = tc.nc
    fp32 = mybir.dt.float32

    with (
        tc.tile_pool(name="setup", bufs=1) as setup_pool,
        tc.tile_pool(name="xin", bufs=3) as xin_pool,
        tc.tile_pool(name="xbf", bufs=3) as xbf_pool,
        tc.tile_pool(name="osb", bufs=3) as osb_pool,
        tc.tile_pool(name="pt", bufs=4, space="PSUM") as pt_pool,
        tc.tile_pool(name="po", bufs=4, space="PSUM") as po_pool,
    ):
        # --- one-time setup: weights, bias, identity, xT double-buffer ---
        wstage = setup_pool.tile([C, K * F], fp32)
        for k in range(K):
            nc.sync.dma_start(out=wstage[:, k * F : (k + 1) * F], in_=w_ap[k])
        bstage = setup_pool.tile([1, F], fp32)
        nc.sync.dma_start(out=bstage[:, :], in_=b_ap[None, :])

        # w rows 0-63 = w[k]; row 64 of the k=0 slice = bias; rows 65-127
        # zero (pad to K=128 -> measurably faster matmuls).
        wpad = setup_pool.tile([128, K * F], mm_dt)
        nc.vector.memset(wpad[:, :], 0.0)
        nc.vector.tensor_copy(wpad[0:C, :], wstage[:, :])
        nc.vector.tensor_copy(wpad[C : C + 1, 0:F], bstage[:, :])

        ident = setup_pool.tile([128, 128], mm_dt)
        make_identity(nc, ident)

        # xT: manually double-buffered [128, 2*L]; row 64 ones and zero pad
        # rows 65-127 (K=128 matmuls). Pad rows are initialized piecewise
        # alongside the first two batches' fills so they don't gate startup.
        xT = setup_pool.tile([128, 2 * L], mm_dt)

        for b in range(B_SHARD):
            half = (b % 2) * L

            def fill_chunk(b, ci, half):
                c0 = ci * IN_CHUNK
                xin = xin_pool.tile([128, T_IN * C], fp32, name=f"xin_{b}_{ci}", tag="xin")
                ieng = nc.sync if (b * 8 + ci) % 2 == 0 else nc.scalar
                ieng.dma_start(
                    out=xin.rearrange("p (t c) -> p t c", c=C),
                    in_=x_ap[b, c0 : c0 + IN_CHUNK, :].rearrange(
                        "(t p) c -> p t c", p=128
                    ),
                )
                if b < 2:
                    c0h = half + c0
                    nc.vector.memset(xT[C:128, c0h : c0h + IN_CHUNK], 0.0)
                    nc.gpsimd.memset(xT[C : C + 1, c0h : c0h + IN_CHUNK], 1.0)
                xbf = xbf_pool.tile([128, T_IN * C], mm_dt, name=f"xbf_{b}_{ci}", tag="xbf")
                nc.gpsimd.tensor_copy(xbf[:, :], xin[:, :])
                # transpose [128,128] sub-tiles; sub-tile j holds t in {2j, 2j+1}
                for j in range(T_IN * C // 128):
                    pt = pt_pool.tile([128, 128], mm_dt, name=f"pt_{b}_{ci}_{j}", tag="pt")
                    nc.tensor.transpose(pt[:, :], xbf[:, j * 128 : (j + 1) * 128], ident)
                    for tt in range(2):
                        q = half + c0 + (2 * j + tt) * 128
                        nc.vector.tensor_copy(
                            xT[0:C, q : q + 128],
                            pt[tt * C : (tt + 1) * C, :],
                        )

            def store_chunk(b, oc, half):
                o0 = oc * OUT_CHUNK
                opos = min(OUT_CHUNK, L_OUT - o0)  # 2048 or 2046
                osb = osb_pool.tile([128, OUT_CHUNK], fp32, name=f"osb_{b}_{oc}", tag="osb")
                n_g = (opos + G - 1) // G
                for gc in range(n_g):
                    g0 = o0 + gc * G
                    gpos = min(G, L_OUT - g0)  # 512 or 510
                    po = po_pool.tile([128, G], fp32, name=f"po_{b}_{oc}_{gc}", tag="po")
                    n_sub = (gpos + 127) // 128
                    for t in range(n_sub):
                        p0 = g0 + t * 128
                        P = min(128, L_OUT - p0)
                        for k in range(K):
                            nc.tensor.matmul(
                                po[0:P, t * F : (t + 1) * F],
                                xT[:, half + p0 + k : half + p0 + k + P],
                                wpad[:, k * F : (k + 1) * F],
                                start=(k == 0),
                                stop=(k == K - 1),
                            )
                    ob = gc * G
                    full_sub = gpos // 128
                    tail_sub = gpos - full_sub * 128
                    if full_sub:
                        nc.scalar.activation(
                            osb[:, ob : ob + full_sub * F],
                            po[:, 0 : full_sub * F],
                            mybir.ActivationFunctionType.Relu,
                        )
                    if tail_sub:
                        nc.scalar.activation(
                            osb[0:tail_sub, ob + full_sub * F : ob + n_sub * F],
                            po[0:tail_sub, full_sub * F : n_sub * F],
                            mybir.ActivationFunctionType.Relu,
                        )
                # store: full tiles in one big DMA, tail tile separately;
                # alternate HWDGE rings so both drain outputs in parallel
                eng = nc.scalar if (b * 4 + oc) % 2 == 0 else nc.sync
                n_full = opos // 128
                tail = opos - n_full * 128
                if n_full:
                    eng.dma_start(
                        out=out_ap[b, o0 : o0 + n_full * 128, :].rearrange(
                            "(t p) f -> p t f", p=128
                        ),
                        in_=osb[:, 0 : n_full * F].rearrange("p (t f) -> p t f", f=F),
                    )
                if tail:
                    eng.dma_start(
                        out=out_ap[b, o0 + n_full * 128 : o0 + opos, :],
                        in_=osb[0:tail, n_full * F : (n_full + 1) * F],
                    )

            for ci in range(L // IN_CHUNK):
                fill_chunk(b, ci, half)
                if ci >= 1:
                    store_chunk(b, ci - 1, half)
            store_chunk(b, L // IN_CHUNK - 1, half)

def build_program(mm_dt=MM_DT):
    nc = bacc.Bacc("TRN2", target_bir_lowering=False, debug=False)
    x = nc.dram_tensor("x", [B_SHARD, L, C], mybir.dt.float32, kind="ExternalInput")
    w = nc.dram_tensor("w", [K, C, F], mybir.dt.float32, kind="ExternalInput")
    bb = nc.dram_tensor("b", [F], mybir.dt.float32, kind="ExternalInput")
    out = nc.dram_tensor(
        "out", [B_SHARD, L_OUT, F], mybir.dt.float32, kind="ExternalOutput"
    )
    with tile.TileContext(nc) as tc:
        _conv_kernel(tc, out.ap(), x.ap(), w.ap(), bb.ap(), mm_dt)
    nc.compile()
    return nc


def kernel(x, w, b, _trace=False, _trace_kwargs=None):
    x = np.ascontiguousarray(np.asarray(x, dtype=np.float32))
    w = np.ascontiguousarray(np.asarray(w, dtype=np.float32))
    b = np.ascontiguousarray(np.asarray(b, dtype=np.float32))
    assert x.shape == (B, L, C) and w.shape == (K, C, F) and b.shape == (F,)

    nc = build_program()
    in_maps = [
        {"x": x[i * B_SHARD : (i + 1) * B_SHARD], "w": w, "b": b}
        for i in range(N_CORES)
    ]
    res = run_bass_kernel_spmd(
        nc,
        in_maps,
        core_ids=list(range(N_CORES)),
        trace=_trace,
        **(_trace_kwargs or {}),
    )
    out = np.concatenate([r["out"] for r in res.results], axis=0)
    if _trace:
        return out, res
    return out


if __name__ == "__main__":
    rng = np.random.default_rng(0)
    x = rng.standard_normal((B, L, C), dtype=np.float32)
    w = rng.standard_normal((K, C, F), dtype=np.float32) * 0.08
    b = np.zeros((F,), dtype=np.float32)
    out = kernel(x, w, b)
    print("out", out.shape, out.dtype, float(np.abs(out).max()))


# revision 29
# speedup vs baseline: 1.2458x; 1.1075x over previous
"""Conv1D (B=32, L=8192, C_in=64, K=3, F=128, VALID) + bias + ReLU on 8 trn2 cores.

Strategy: data-parallel over batch (4 batches per core). Per core:
  - Casting SWDGE DMA loads x[b] position-chunks as bf16, t-major
    ([128, T*64] tiles; partition = position within each 128-block).
  - PE-transpose [128,128] bf16 sub-tiles; both PSUM halves copy out as
    contiguous [64,128] blocks into x_T [65, L] (row 64 = ones so the bias
    rides row 64 of the k=0 weights).
  - out[pos, F]: per 512-position PSUM bank, 4 interleaved-M matmul groups
    (position = g0 + 4*q + m) x 3 accumulated k-shifts; interleaving makes
    the output DMA's per-partition DRAM runs 2KB instead of 512B.
  - One ReLU (ScalarE) per bank -> SBUF staging -> big contiguous DMA out.
"""

import os
import sys

import numpy as np

_TRN_REPO = "/opt/trn_rl_repo"
if _TRN_REPO not in sys.path and os.path.isdir(_TRN_REPO):
    sys.path.insert(0, _TRN_REPO)

import concourse.bass as bass
import concourse.tile as tile
from concourse import bacc, mybir
from concourse.bass_utils import run_bass_kernel_spmd
from concourse.masks import make_identity

B, L, C = 32, 8192, 64
K, F = 3, 128
L_OUT = L - K + 1  # 8190
N_CORES = 8
B_SHARD = B // N_CORES  # 4

MM_DT = mybir.dt.bfloat16

IN_CHUNK = 1024  # positions per input DMA chunk
T_IN = IN_CHUNK // 128  # 16
G = 512  # positions per PSUM output bank
GI = 4  # M-interleave within a bank (G // 128)
OUT_CHUNK = 1024  # positions per output staging tile


def _conv_kernel(tc: tile.TileContext, out_ap, x_ap, w_ap, b_ap, mm_dt):
    nc = tc.nc
    fp32 = mybir.dt.float32

    with (
        tc.tile_pool(name="setup", bufs=1) as setup_pool,
        tc.tile_pool(name="xin", bufs=3) as xin_pool,
        tc.tile_pool(name="xbf", bufs=3) as xbf_pool,
        tc.tile_pool(name="osb", bufs=3) as osb_pool,
        tc.tile_pool(name="pt", bufs=4, space="PSUM") as pt_pool,
        tc.tile_pool(name="po", bufs=4, space="PSUM") as po_pool,
    ):
        # --- one-time setup: weights, bias, identity, xT double-buffer ---
        wstage = setup_pool.tile([C, K * F], fp32)
        for k in range(K):
            nc.sync.dma_start(out=wstage[:, k * F : (k + 1) * F], in_=w_ap[k])
        bstage = setup_pool.tile([1, F], fp32)
        nc.sync.dma_start(out=bstage[:, :], in_=b_ap[None, :])

        # w rows 0-63 = w[k]; row 64 of the k=0 slice = bias; rows 65-127
        # zero (pad to K=128 -> measurably faster matmuls).
        wpad = setup_pool.tile([128, K * F], mm_dt)
        nc.vector.memset(wpad[:, :], 0.0)
        nc.vector.tensor_copy(wpad[0:C, :], wstage[:, :])
        nc.vector.tensor_copy(wpad[C : C + 1, 0:F], bstage[:, :])

        ident = setup_pool.tile([128, 128], mm_dt)
        make_identity(nc, ident)

        # xT: manually double-buffered [128, 2*L]; row 64 ones and zero pad
        # rows 65-127 (K=128 matmuls). Pad rows are initialized piecewise
        # alongside the first two batches' fills so they don't gate startup.
        xT = setup_pool.tile([128, 2 * L], mm_dt)

        for b in range(B_SHARD):
            half = (b % 2) * L

            def fill_chunk(b, ci, half):
                c0 = ci * IN_CHUNK
                xin = xin_pool.tile([128, T_IN * C], fp32, name=f"xin_{b}_{ci}", tag="xin")
                nc.sync.dma_start(
                    out=xin.rearrange("p (t c) -> p t c", c=C),
                    in_=x_ap[b, c0 : c0 + IN_CHUNK, :].rearrange(
                        "(t p) c -> p t c", p=128
                    ),
                )
                if b < 2:
                    c0h = half + c0
                    nc.vector.memset(xT[C:128, c0h : c0h + IN_CHUNK], 0.0)
                    nc.gpsimd.memset(xT[C : C + 1, c0h : c0h + IN_CHUNK], 1.0)
                xbf = xbf_pool.tile([128, T_IN * C], mm_dt, name=f"xbf_{b}_{ci}", tag="xbf")
                nc.gpsimd.tensor_copy(xbf[:, :], xin[:, :])
                # transpose [128,128] sub-tiles; sub-tile j holds t in {2j, 2j+1}
                for j in range(T_IN * C // 128):
                    pt = pt_pool.tile([128, 128], mm_dt, name=f"pt_{b}_{ci}_{j}", tag="pt")
                    nc.tensor.transpose(pt[:, :], xbf[:, j * 128 : (j + 1) * 128], ident)
                    for tt in range(2):
                        q = half + c0 + (2 * j + tt) * 128
                        nc.vector.tensor_copy(
                            xT[0:C, q : q + 128],
                            pt[tt * C : (tt + 1) * C, :],
                        )

            def store_chunk(b, oc, half):
                o0 = oc * OUT_CHUNK
                opos = min(OUT_CHUNK, L_OUT - o0)  # 2048 or 2046
                osb = osb_pool.tile([128, OUT_CHUNK], fp32, name=f"osb_{b}_{oc}", tag="osb")
                n_g = (opos + G - 1) // G
                for gc in range(n_g):
                    g0 = o0 + gc * G
                    gpos = min(G, L_OUT - g0)  # 512 or 510
                    po = po_pool.tile([128, G], fp32, name=f"po_{b}_{oc}_{gc}", tag="po")
                    n_sub = (gpos + 127) // 128
                    for t in range(n_sub):
                        p0 = g0 + t * 128
                        P = min(128, L_OUT - p0)
                        for k in range(K):
                            nc.tensor.matmul(
                                po[0:P, t * F : (t + 1) * F],
                                xT[:, half + p0 + k : half + p0 + k + P],
                                wpad[:, k * F : (k + 1) * F],
                                start=(k == 0),
                                stop=(k == K - 1),
                            )
                    ob = gc * G
                    full_sub = gpos // 128
                    tail_sub = gpos - full_sub * 128
                    if full_sub:
                        nc.scalar.activation(
                            osb[:, ob : ob + full_sub * F],
                            po[:, 0 : full_sub * F],
                            mybir.ActivationFunctionType.Relu,
                        )
                    if tail_sub:
                        nc.scalar.activation(
                            osb[0:tail_sub, ob + full_sub * F : ob + n_sub * F],
                            po[0:tail_sub, full_sub * F : n_sub * F],
                            mybir.ActivationFunctionType.Relu,
                        )
                # store: full tiles in one big DMA, tail tile separately;
                # alternate HWDGE rings so both drain outputs in parallel
                eng = nc.scalar
                n_full = opos // 128
                tail = opos - n_full * 128
                if n_full:
                    eng.dma_start(
                        out=out_ap[b, o0 : o0 + n_full * 128, :].rearrange(
                            "(t p) f -> p t f", p=128
                        ),
                        in_=osb[:, 0 : n_full * F].rearrange("p (t f) -> p t f", f=F),
                    )
                if tail:
                    eng.dma_start(
                        out=out_ap[b, o0 + n_full * 128 : o0 + opos, :],
                        in_=osb[0:tail, n_full * F : (n_full + 1) * F],
                    )

            for ci in range(L // IN_CHUNK):
                fill_chunk(b, ci, half)
                if ci >= 1:
                    store_chunk(b, ci - 1, half)
            store_chunk(b, L // IN_CHUNK - 1, half)

def build_program(mm_dt=MM_DT):
    nc = bacc.Bacc("TRN2", target_bir_lowering=False, debug=False)
    x = nc.dram_tensor("x", [B_SHARD, L, C], mybir.dt.float32, kind="ExternalInput")
    w = nc.dram_tensor("w", [K, C, F], mybir.dt.float32, kind="ExternalInput")
    bb = nc.dram_tensor("b", [F], mybir.dt.float32, kind="ExternalInput")
    out = nc.dram_tensor(
        "out", [B_SHARD, L_OUT, F], mybir.dt.float32, kind="ExternalOutput"
    )
    with tile.TileContext(nc) as tc:
        _conv_kernel(tc, out.ap(), x.ap(), w.ap(), bb.ap(), mm_dt)
    nc.compile()
    return nc


def kernel(x, w, b, _trace=False, _trace_kwargs=None):
    x = np.ascontiguousarray(np.asarray(x, dtype=np.float32))
    w = np.ascontiguousarray(np.asarray(w, dtype=np.float32))
    b = np.ascontiguousarray(np.asarray(b, dtype=np.float32))
    assert x.shape == (B, L, C) and w.shape == (K, C, F) and b.shape == (F,)

    nc = build_program()
    in_maps = [
        {"x": x[i * B_SHARD : (i + 1) * B_SHARD], "w": w, "b": b}
        for i in range(N_CORES)
    ]
    res = run_bass_kernel_spmd(
        nc,
        in_maps,
        core_ids=list(range(N_CORES)),
        trace=_trace,
        **(_trace_kwargs or {}),
    )
    out = np.concatenate([r["out"] for r in res.results], axis=0)
    if _trace:
        return out, res
    return out


if __name__ == "__main__":
    rng = np.random.default_rng(0)
    x = rng.standard_normal((B, L, C), dtype=np.float32)
    w = rng.standard_normal((K, C, F), dtype=np.float32) * 0.08
    b = np.zeros((F,), dtype=np.float32)
    out = kernel(x, w, b)
    print("out", out.shape, out.dtype, float(np.abs(out).max()))


# revision 30
# speedup vs baseline: 1.4126x; 1.1339x over previous
"""Conv1D (B=32, L=8192, C_in=64, K=3, F=128, VALID) + bias + ReLU on 8 trn2 cores.

Strategy: data-parallel over batch (4 batches per core). Per core:
  - Casting SWDGE DMA loads x[b] position-chunks as bf16, t-major
    ([128, T*64] tiles; partition = position within each 128-block).
  - PE-transpose [128,128] bf16 sub-tiles; both PSUM halves copy out as
    contiguous [64,128] blocks into x_T [65, L] (row 64 = ones so the bias
    rides row 64 of the k=0 weights).
  - out[pos, F]: per 512-position PSUM bank, 4 interleaved-M matmul groups
    (position = g0 + 4*q + m) x 3 accumulated k-shifts; interleaving makes
    the output DMA's per-partition DRAM runs 2KB instead of 512B.
  - One ReLU (ScalarE) per bank -> SBUF staging -> big contiguous DMA out.
"""

import os
import sys

import numpy as np

_TRN_REPO = "/opt/trn_rl_repo"
if _TRN_REPO not in sys.path and os.path.isdir(_TRN_REPO):
    sys.path.insert(0, _TRN_REPO)

import concourse.bass as bass
import concourse.tile as tile
from concourse import bacc, mybir
from concourse.bass_utils import run_bass_kernel_spmd
from concourse.masks import make_identity

B, L, C = 32, 8192, 64
K, F = 3, 128
L_OUT = L - K + 1  # 8190
N_CORES = 8
B_SHARD = B // N_CORES  # 4

MM_DT = mybir.dt.bfloat16

IN_CHUNK = 1024  # positions per input DMA chunk
T_IN = IN_CHUNK // 128  # 16
G = 512  # positions per PSUM output bank
GI = 4  # M-interleave within a bank (G // 128)
OUT_CHUNK = 1024  # positions per output staging tile


def _conv_kernel(tc: tile.TileContext, out_ap, x_ap, w_ap, b_ap, mm_dt):
    nc = tc.nc
    fp32 = mybir.dt.float32

    with (
        tc.tile_pool(name="setup", bufs=1) as setup_pool,
        tc.tile_pool(name="xin", bufs=3) as xin_pool,
        tc.tile_pool(name="xbf", bufs=3) as xbf_pool,
        tc.tile_pool(name="osb", bufs=3) as osb_pool,
        tc.tile_pool(name="pt", bufs=4, space="PSUM") as pt_pool,
        tc.tile_pool(name="po", bufs=4, space="PSUM") as po_pool,
    ):
        # --- one-time setup: weights, bias, identity, xT double-buffer ---
        wstage = setup_pool.tile([C, K * F], fp32)
        for k in range(K):
            nc.sync.dma_start(out=wstage[:, k * F : (k + 1) * F], in_=w_ap[k])
        bstage = setup_pool.tile([1, F], fp32)
        nc.sync.dma_start(out=bstage[:, :], in_=b_ap[None, :])

        # w rows 0-63 = w[k]; row 64 of the k=0 slice = bias; rows 65-127
        # zero (pad to K=128 -> measurably faster matmuls).
        wpad = setup_pool.tile([128, K * F], mm_dt)
        nc.vector.memset(wpad[:, :], 0.0)
        nc.vector.tensor_copy(wpad[0:C, :], wstage[:, :])
        nc.vector.tensor_copy(wpad[C : C + 1, 0:F], bstage[:, :])

        ident = setup_pool.tile([128, 128], mm_dt)
        make_identity(nc, ident)

        # xT: manually double-buffered [128, 2*L]; row 64 ones and zero pad
        # rows 65-127 (K=128 matmuls). Pad rows are initialized piecewise
        # alongside the first two batches' fills so they don't gate startup.
        xT = setup_pool.tile([128, 2 * L], mm_dt)

        for b in range(B_SHARD):
            half = (b % 2) * L

            def fill_chunk(b, ci, half):
                c0 = ci * IN_CHUNK
                xin = xin_pool.tile([128, T_IN * C], fp32, name=f"xin_{b}_{ci}", tag="xin")
                nc.sync.dma_start(
                    out=xin.rearrange("p (t c) -> p t c", c=C),
                    in_=x_ap[b, c0 : c0 + IN_CHUNK, :].rearrange(
                        "(t p) c -> p t c", p=128
                    ),
                )
                if b < 2:
                    c0h = half + c0
                    nc.vector.memset(xT[C:128, c0h : c0h + IN_CHUNK], 0.0)
                    nc.gpsimd.memset(xT[C : C + 1, c0h : c0h + IN_CHUNK], 1.0)
                xbf = xbf_pool.tile([128, T_IN * C], mm_dt, name=f"xbf_{b}_{ci}", tag="xbf")
                if ci % 2 == 0:
                    nc.gpsimd.tensor_copy(xbf[:, :], xin[:, :])
                else:
                    nc.scalar.copy(xbf[:, :], xin[:, :])
                # transpose [128,128] sub-tiles; sub-tile j holds t in {2j, 2j+1}
                for j in range(T_IN * C // 128):
                    pt = pt_pool.tile([128, 128], mm_dt, name=f"pt_{b}_{ci}_{j}", tag="pt")
                    nc.tensor.transpose(pt[:, :], xbf[:, j * 128 : (j + 1) * 128], ident)
                    for tt in range(2):
                        q = half + c0 + (2 * j + tt) * 128
                        nc.vector.tensor_copy(
                            xT[0:C, q : q + 128],
                            pt[tt * C : (tt + 1) * C, :],
                        )

            def store_chunk(b, oc, half):
                o0 = oc * OUT_CHUNK
                opos = min(OUT_CHUNK, L_OUT - o0)  # 2048 or 2046
                osb = osb_pool.tile([128, OUT_CHUNK], fp32, name=f"osb_{b}_{oc}", tag="osb")
                n_g = (opos + G - 1) // G
                for gc in range(n_g):
                    g0 = o0 + gc * G
                    gpos = min(G, L_OUT - g0)  # 512 or 510
                    po = po_pool.tile([128, G], fp32, name=f"po_{b}_{oc}_{gc}", tag="po")
                    n_sub = (gpos + 127) // 128
                    for t in range(n_sub):
                        p0 = g0 + t * 128
                        P = min(128, L_OUT - p0)
                        for k in range(K):
                            nc.tensor.matmul(
                                po[0:P, t * F : (t + 1) * F],
                                xT[:, half + p0 + k : half + p0 + k + P],
                                wpad[:, k * F : (k + 1) * F],
                                start=(k == 0),
                                stop=(k == K - 1),
                            )
                    ob = gc * G
                    full_sub = gpos // 128
                    tail_sub = gpos - full_sub * 128
                    if full_sub:
                        nc.scalar.activation(
                            osb[:, ob : ob + full_sub * F],
                            po[:, 0 : full_sub * F],
                            mybir.ActivationFunctionType.Relu,
                        )
                    if tail_sub:
                        nc.scalar.activation(
                            osb[0:tail_sub, ob + full_sub * F : ob + n_sub * F],
                            po[0:tail_sub, full_sub * F : n_sub * F],
                            mybir.ActivationFunctionType.Relu,
                        )
                # store: full tiles in one big DMA, tail tile separately;
                # alternate HWDGE rings so both drain outputs in parallel
                eng = nc.scalar if (b * 4 + oc) % 2 == 0 else nc.sync
                n_full = opos // 128
                tail = opos - n_full * 128
                if n_full:
                    eng.dma_start(
                        out=out_ap[b, o0 : o0 + n_full * 128, :].rearrange(
                            "(t p) f -> p t f", p=128
                        ),
                        in_=osb[:, 0 : n_full * F].rearrange("p (t f) -> p t f", f=F),
                    )
                if tail:
                    eng.dma_start(
                        out=out_ap[b, o0 + n_full * 128 : o0 + opos, :],
                        in_=osb[0:tail, n_full * F : (n_full + 1) * F],
                    )

            for ci in range(L // IN_CHUNK):
                fill_chunk(b, ci, half)
                if ci >= 1:
                    store_chunk(b, ci - 1, half)
            store_chunk(b, L // IN_CHUNK - 1, half)

def build_program(mm_dt=MM_DT):
    nc = bacc.Bacc("TRN2", target_bir_lowering=False, debug=False)
    x = nc.dram_tensor("x", [B_SHARD, L, C], mybir.dt.float32, kind="ExternalInput")
    w = nc.dram_tensor("w", [K, C, F], mybir.dt.float32, kind="ExternalInput")
    bb = nc.dram_tensor("b", [F], mybir.dt.float32, kind="ExternalInput")
    out = nc.dram_tensor(
        "out", [B_SHARD, L_OUT, F], mybir.dt.float32, kind="ExternalOutput"
    )
    with tile.TileContext(nc) as tc:
        _conv_kernel(tc, out.ap(), x.ap(), w.ap(), bb.ap(), mm_dt)
    nc.compile()
    return nc


def kernel(x, w, b, _trace=False, _trace_kwargs=None):
    x = np.ascontiguousarray(np.asarray(x, dtype=np.float32))
    w = np.ascontiguousarray(np.asarray(w, dtype=np.float32))
    b = np.ascontiguousarray(np.asarray(b, dtype=np.float32))
    assert x.shape == (B, L, C) and w.shape == (K, C, F) and b.shape == (F,)

    nc = build_program()
    in_maps = [
        {"x": x[i * B_SHARD : (i + 1) * B_SHARD], "w": w, "b": b}
        for i in range(N_CORES)
    ]
    res = run_bass_kernel_spmd(
        nc,
        in_maps,
        core_ids=list(range(N_CORES)),
        trace=_trace,
        **(_trace_kwargs or {}),
    )
    out = np.concatenate([r["out"] for r in res.results], axis=0)
    if _trace:
        return out, res
    return out


if __name__ == "__main__":
    rng = np.random.default_rng(0)
    x = rng.standard_normal((B, L, C), dtype=np.float32)
    w = rng.standard_normal((K, C, F), dtype=np.float32) * 0.08
    b = np.zeros((F,), dtype=np.float32)
    out = kernel(x, w, b)
    print("out", out.shape, out.dtype, float(np.abs(out).max()))


# revision 31
# speedup vs baseline: 1.4189x; 1.0045x over previous
"""Conv1D (B=32, L=8192, C_in=64, K=3, F=128, VALID) + bias + ReLU on 8 trn2 cores.

Strategy: data-parallel over batch (4 batches per core). Per core:
  - Casting SWDGE DMA loads x[b] position-chunks as bf16, t-major
    ([128, T*64] tiles; partition = position within each 128-block).
  - PE-transpose [128,128] bf16 sub-tiles; both PSUM halves copy out as
    contiguous [64,128] blocks into x_T [65, L] (row 64 = ones so the bias
    rides row 64 of the k=0 weights).
  - out[pos, F]: per 512-position PSUM bank, 4 interleaved-M matmul groups
    (position = g0 + 4*q + m) x 3 accumulated k-shifts; interleaving makes
    the output DMA's per-partition DRAM runs 2KB instead of 512B.
  - One ReLU (ScalarE) per bank -> SBUF staging -> big contiguous DMA out.
"""

import os
import sys

import numpy as np

_TRN_REPO = "/opt/trn_rl_repo"
if _TRN_REPO not in sys.path and os.path.isdir(_TRN_REPO):
    sys.path.insert(0, _TRN_REPO)

import concourse.bass as bass
import concourse.tile as tile
from concourse import bacc, mybir
from concourse.bass_utils import run_bass_kernel_spmd
from concourse.masks import make_identity

B, L, C = 32, 8192, 64
K, F = 3, 128
L_OUT = L - K + 1  # 8190
N_CORES = 8
B_SHARD = B // N_CORES  # 4

MM_DT = mybir.dt.bfloat16

IN_CHUNK = 1024  # positions per input DMA chunk
T_IN = IN_CHUNK // 128  # 16
G = 512  # positions per PSUM output bank
GI = 4  # M-interleave within a bank (G // 128)
OUT_CHUNK = 1024  # positions per output staging tile


def _conv_kernel(tc: tile.TileContext, out_ap, x_ap, w_ap, b_ap, mm_dt):
    nc = tc.nc
    fp32 = mybir.dt.float32

    with (
        tc.tile_pool(name="setup", bufs=1) as setup_pool,
        tc.tile_pool(name="xin", bufs=3) as xin_pool,
        tc.tile_pool(name="xbf", bufs=3) as xbf_pool,
        tc.tile_pool(name="osb", bufs=3) as osb_pool,
        tc.tile_pool(name="pt", bufs=3, space="PSUM") as pt_pool,
        tc.tile_pool(name="po", bufs=5, space="PSUM") as po_pool,
    ):
        # --- one-time setup: weights, bias, identity, xT double-buffer ---
        wstage = setup_pool.tile([C, K * F], fp32)
        for k in range(K):
            nc.sync.dma_start(out=wstage[:, k * F : (k + 1) * F], in_=w_ap[k])
        bstage = setup_pool.tile([1, F], fp32)
        nc.sync.dma_start(out=bstage[:, :], in_=b_ap[None, :])

        # w rows 0-63 = w[k]; row 64 of the k=0 slice = bias; rows 65-127
        # zero (pad to K=128 -> measurably faster matmuls).
        wpad = setup_pool.tile([128, K * F], mm_dt)
        nc.vector.memset(wpad[:, :], 0.0)
        nc.vector.tensor_copy(wpad[0:C, :], wstage[:, :])
        nc.vector.tensor_copy(wpad[C : C + 1, 0:F], bstage[:, :])

        ident = setup_pool.tile([128, 128], mm_dt)
        make_identity(nc, ident)

        # xT: manually double-buffered [128, 2*L]; row 64 ones and zero pad
        # rows 65-127 (K=128 matmuls). Pad rows are initialized piecewise
        # alongside the first two batches' fills so they don't gate startup.
        xT = setup_pool.tile([128, 2 * L], mm_dt)

        for b in range(B_SHARD):
            half = (b % 2) * L

            def fill_chunk(b, ci, half):
                c0 = ci * IN_CHUNK
                xin = xin_pool.tile([128, T_IN * C], fp32, name=f"xin_{b}_{ci}", tag="xin")
                nc.sync.dma_start(
                    out=xin.rearrange("p (t c) -> p t c", c=C),
                    in_=x_ap[b, c0 : c0 + IN_CHUNK, :].rearrange(
                        "(t p) c -> p t c", p=128
                    ),
                )
                if b < 2:
                    c0h = half + c0
                    nc.vector.memset(xT[C:128, c0h : c0h + IN_CHUNK], 0.0)
                    nc.gpsimd.memset(xT[C : C + 1, c0h : c0h + IN_CHUNK], 1.0)
                xbf = xbf_pool.tile([128, T_IN * C], mm_dt, name=f"xbf_{b}_{ci}", tag="xbf")
                if ci % 2 == 0:
                    nc.gpsimd.tensor_copy(xbf[:, :], xin[:, :])
                else:
                    nc.scalar.copy(xbf[:, :], xin[:, :])
                # transpose [128,128] sub-tiles; sub-tile j holds t in {2j, 2j+1}
                for j in range(T_IN * C // 128):
                    pt = pt_pool.tile([128, 128], mm_dt, name=f"pt_{b}_{ci}_{j}", tag="pt")
                    nc.tensor.transpose(pt[:, :], xbf[:, j * 128 : (j + 1) * 128], ident)
                    for tt in range(2):
                        q = half + c0 + (2 * j + tt) * 128
                        nc.vector.tensor_copy(
                            xT[0:C, q : q + 128],
                            pt[tt * C : (tt + 1) * C, :],
                        )

            def store_chunk(b, oc, half):
                o0 = oc * OUT_CHUNK
                opos = min(OUT_CHUNK, L_OUT - o0)  # 2048 or 2046
                osb = osb_pool.tile([128, OUT_CHUNK], fp32, name=f"osb_{b}_{oc}", tag="osb")
                n_g = (opos + G - 1) // G
                for gc in range(n_g):
                    g0 = o0 + gc * G
                    gpos = min(G, L_OUT - g0)  # 512 or 510
                    po = po_pool.tile([128, G], fp32, name=f"po_{b}_{oc}_{gc}", tag="po")
                    n_sub = (gpos + 127) // 128
                    for t in range(n_sub):
                        p0 = g0 + t * 128
                        P = min(128, L_OUT - p0)
                        for k in range(K):
                            nc.tensor.matmul(
                                po[0:P, t * F : (t + 1) * F],
                                xT[:, half + p0 + k : half + p0 + k + P],
                                wpad[:, k * F : (k + 1) * F],
                                start=(k == 0),
                                stop=(k == K - 1),
                            )
                    ob = gc * G
                    full_sub = gpos // 128
                    tail_sub = gpos - full_sub * 128
                    if full_sub:
                        nc.scalar.activation(
                            osb[:, ob : ob + full_sub * F],
                            po[:, 0 : full_sub * F],
                            mybir.ActivationFunctionType.Relu,
                        )
                    if tail_sub:
                        nc.scalar.activation(
                            osb[0:tail_sub, ob + full_sub * F : ob + n_sub * F],
                            po[0:tail_sub, full_sub * F : n_sub * F],
                            mybir.ActivationFunctionType.Relu,
                        )
                # store: full tiles in one big DMA, tail tile separately;
                # alternate HWDGE rings so both drain outputs in parallel
                eng = nc.scalar if (b * 4 + oc) % 2 == 0 else nc.sync
                n_full = opos // 128
                tail = opos - n_full * 128
                if n_full:
                    eng.dma_start(
                        out=out_ap[b, o0 : o0 + n_full * 128, :].rearrange(
                            "(t p) f -> p t f", p=128
                        ),
                        in_=osb[:, 0 : n_full * F].rearrange("p (t f) -> p t f", f=F),
                    )
                if tail:
                    eng.dma_start(
                        out=out_ap[b, o0 + n_full * 128 : o0 + opos, :],
                        in_=osb[0:tail, n_full * F : (n_full + 1) * F],
                    )

            for ci in range(L // IN_CHUNK):
                fill_chunk(b, ci, half)
                if ci >= 1:
                    store_chunk(b, ci - 1, half)
            store_chunk(b, L // IN_CHUNK - 1, half)

def build_program(mm_dt=MM_DT):
    nc = bacc.Bacc("TRN2", target_bir_lowering=False, debug=False)
    x = nc.dram_tensor("x", [B_SHARD, L, C], mybir.dt.float32, kind="ExternalInput")
    w = nc.dram_tensor("w", [K, C, F], mybir.dt.float32, kind="ExternalInput")
    bb = nc.dram_tensor("b", [F], mybir.dt.float32, kind="ExternalInput")
    out = nc.dram_tensor(
        "out", [B_SHARD, L_OUT, F], mybir.dt.float32, kind="ExternalOutput"
    )
    with tile.TileContext(nc) as tc:
        _conv_kernel(tc, out.ap(), x.ap(), w.ap(), bb.ap(), mm_dt)
    nc.compile()
    return nc


def kernel(x, w, b, _trace=False, _trace_kwargs=None):
    x = np.ascontiguousarray(np.asarray(x, dtype=np.float32))
    w = np.ascontiguousarray(np.asarray(w, dtype=np.float32))
    b = np.ascontiguousarray(np.asarray(b, dtype=np.float32))
    assert x.shape == (B, L, C) and w.shape == (K, C, F) and b.shape == (F,)

    nc = build_program()
    in_maps = [
        {"x": x[i * B_SHARD : (i + 1) * B_SHARD], "w": w, "b": b}
        for i in range(N_CORES)
    ]
    res = run_bass_kernel_spmd(
        nc,
        in_maps,
        core_ids=list(range(N_CORES)),
        trace=_trace,
        **(_trace_kwargs or {}),
    )
    out = np.concatenate([r["out"] for r in res.results], axis=0)
    if _trace:
        return out, res
    return out


if __name__ == "__main__":
    rng = np.random.default_rng(0)
    x = rng.standard_normal((B, L, C), dtype=np.float32)
    w = rng.standard_normal((K, C, F), dtype=np.float32) * 0.08
    b = np.zeros((F,), dtype=np.float32)
    out = kernel(x, w, b)
    print("out", out.shape, out.dtype, float(np.abs(out).max()))
